# revision 23
# baseline (speedup 1.0000x reference)
"""DeltaNet single-token decode step on 8 Trainium2 NeuronCores.

Sharding: tensor-parallel over the 32 delta-rule heads. Core c owns heads
4c..4c+3 (which share q/k heads 2c and 2c+1), the matching 1024 rows of
W_qkv, 512 rows of W_z, the matching 1024 depthwise-conv channels, the
4 state slabs, and columns 512c..512c+512 of W_out. Each core produces a
partial out-projection (summed on host), its new-state shard and its
new-conv shard. No device collectives are needed.

The kernel is memory bound: ~16.5 MiB of weights stream through each
core exactly once (~48 us of DMA at the ~360 GB/s per-core HBM rate), so
the mat-vec work is split across three engines to keep each under that
floor (fp32 PE matmul streams at ~1/4 the bf16 rate, so PE alone cannot):
  - W_qkv (8 MiB) runs as DVE multiply + ACT Copy-accumulate (the
    working fused-reduce on this runtime) against a partition-broadcast
    x; mixed lands as per-channel columns, which makes the depthwise
    conv and silu cheap [128,8] column ops.
  - W_z (2 MiB) and half of W_out run on the tensor engine with
    host-transposed chunks and x / y as stationary [128,1] columns.
  - The other half of W_out runs on DVE+ACT, so the final reduction
    tail is split across engines.
  - rsqrt is a DVE Newton iteration (quake seed; no ACT tables, no slow
    DVE reciprocal). Sums of squares use PE Gram mat-muls / ACT
    Square-accumulate. Square/Copy live in every ACT table set and the
    real transcendentals (Exp/Ln/Sigmoid/Silu) are all first issued
    early (the a/b projection uses its own tiny weights, done in the
    first microseconds), so no table-set load lands on the critical path.
"""

import numpy as np

import concourse.bacc as bacc
import concourse.bass as bass
import concourse.tile as tile
from concourse import mybir
from concourse.bass_utils import run_bass_kernel_spmd

F32 = mybir.dt.float32
I32 = mybir.dt.int32
AX = mybir.AxisListType
OP = mybir.AluOpType
ACT = mybir.ActivationFunctionType

N_CORES = 8
H = 2048
EPS = 1e-6
QSCALE = float(1.0 / np.sqrt(128.0))
MAGIC = 0x5F3759DF


def _rep2(ap):
    """View a [1, n] AP as [1, n, 2] reading each element twice (free step 0)."""
    return bass.AP(tensor=ap.tensor, offset=ap.offset, ap=list(ap.ap) + [[0, 2]])


def _bcast_part(ap, p):
    """View an AP as having p partitions with partition step 0 (for DMA)."""
    return bass.AP(tensor=ap.tensor, offset=ap.offset, ap=[[0, p]] + list(ap.ap))


def build_nc():
    nc = bacc.Bacc("TRN2", target_bir_lowering=False, debug=False)

    # inputs (per-core shards; layouts prearranged on host)
    x_t = nc.dram_tensor("x", [16, 128], F32, kind="ExternalInput")
    wq_t = nc.dram_tensor("wq", [8, 128, H], F32, kind="ExternalInput")
    wzp_t = nc.dram_tensor("wzT", [8, 128, 512], F32, kind="ExternalInput")
    wzn_t = nc.dram_tensor("wzn", [4, 128, 1024], F32, kind="ExternalInput")
    wab_t = nc.dram_tensor("wabT", [128, 16, 8], F32, kind="ExternalInput")
    wo1_t = nc.dram_tensor("wo1", [128, 12, 512], F32, kind="ExternalInput")
    wo2_t = nc.dram_tensor("wo2T", [4, 128, 512], F32, kind="ExternalInput")
    cs_t = nc.dram_tensor("cs", [128, 8, 3], F32, kind="ExternalInput")
    cw_t = nc.dram_tensor("cw", [128, 8, 4], F32, kind="ExternalInput")
    st_t = nc.dram_tensor("st", [4, 128, 128], F32, kind="ExternalInput")
    al_t = nc.dram_tensor("alog", [1, 4], F32, kind="ExternalInput")
    db_t = nc.dram_tensor("dtb", [1, 4], F32, kind="ExternalInput")
    nw_t = nc.dram_tensor("nw", [1, 128], F32, kind="ExternalInput")
    id_t = nc.dram_tensor("ident", [128, 128], F32, kind="ExternalInput")
    on_t = nc.dram_tensor("onesr", [1, 128], F32, kind="ExternalInput")

    # outputs
    o1_t = nc.dram_tensor("out_p1", [128, 12], F32, kind="ExternalOutput")
    o2_t = nc.dram_tensor("out_p2", [1, 512], F32, kind="ExternalOutput")
    ns_t = nc.dram_tensor("nst", [4, 128, 128], F32, kind="ExternalOutput")
    nv_t = nc.dram_tensor("ncv", [128, 8, 3], F32, kind="ExternalOutput")

    with (
        tile.TileContext(nc) as tc,
        tc.tile_pool(name="consts", bufs=1) as consts,
        tc.tile_pool(name="wpool", bufs=4) as wpool,
        tc.tile_pool(name="rows", bufs=2) as rowp,
        tc.tile_pool(name="psA", bufs=1, space="PSUM") as psA,
        tc.tile_pool(name="psB", bufs=2, space="PSUM") as psB,
        tc.tile_pool(name="psC", bufs=3, space="PSUM") as psC,
    ):
        # ---- constants / setup ----------------------------------------
        ident = consts.tile([128, 128], F32)
        nc.gpsimd.dma_start(out=ident, in_=id_t[:])
        ones_r = consts.tile([1, 128], F32)
        nc.gpsimd.dma_start(out=ones_r, in_=on_t[:])

        # x: [16,128] rows -> one transpose -> [128,16] stationary columns
        x16 = consts.tile([16, 128], F32)
        nc.sync.dma_start(out=x16, in_=x_t[:])
        xt_ps = psB.tile([128, 16], F32, tag="tp", padded_shape=[128, 512])
        nc.tensor.transpose(xt_ps, x16, ident[0:16, 0:16])
        x_sb = consts.tile([128, 16], F32)
        nc.vector.tensor_copy(x_sb, xt_ps)

        # x broadcast down all partitions for the DVE mat-vec paths
        x_b = consts.tile([128, H], F32)
        nc.gpsimd.dma_start(
            out=x_b, in_=_bcast_part(x_t[:].rearrange("a b -> (a b)"), 128)
        )

        # small loads
        alr = consts.tile([1, 4], F32)
        nc.gpsimd.dma_start(out=alr, in_=al_t[:])
        dbr = consts.tile([1, 4], F32)
        nc.gpsimd.dma_start(out=dbr, in_=db_t[:])
        nwr = consts.tile([1, 128], F32)
        nc.gpsimd.dma_start(out=nwr, in_=nw_t[:])
        cmb = consts.tile([128, 8, 4], F32)
        nc.gpsimd.dma_start(out=cmb[:, :, 0:3], in_=cs_t[:])
        cwt = consts.tile([128, 8, 4], F32)
        nc.gpsimd.dma_start(out=cwt, in_=cw_t[:])
        s_all = consts.tile([128, 4, 128], F32)
        nc.gpsimd.dma_start(out=s_all, in_=st_t[:].rearrange("i p v -> p i v"))
        wab_sb = consts.tile([128, 16, 8], F32)
        nc.gpsimd.dma_start(out=wab_sb, in_=wab_t[:])

        # ---- a/b projection first (tiny weights): scalars come early ---
        mab = psA.tile([1, 8], F32, tag="b", padded_shape=[1, 512])
        for j in range(16):
            nc.tensor.matmul(
                mab, x_sb[:, j : j + 1], wab_sb[:, j, :],
                start=(j == 0), stop=(j == 15),
            )
        abr = consts.tile([1, 8], F32)
        nc.vector.tensor_copy(abr, mab)
        # softplus(a + dt_bias) = ln(1 + exp(.)) ; exps batched before ln
        sp4 = consts.tile([1, 4], F32)
        nc.vector.tensor_add(sp4, abr[:, 0:4], dbr)
        nc.scalar.activation(sp4, sp4, ACT.Exp)
        ea4 = consts.tile([1, 4], F32)
        nc.scalar.activation(ea4, alr, ACT.Exp)
        nc.vector.tensor_scalar_add(sp4, sp4, 1.0)
        nc.scalar.activation(sp4, sp4, ACT.Ln)
        # g_t = exp(-exp(A_log) * softplus)
        gt4 = consts.tile([1, 4], F32)
        nc.vector.tensor_mul(gt4, ea4, sp4)
        nc.scalar.activation(gt4, gt4, ACT.Exp, scale=-1.0)
        # beta = sigmoid(b)
        bet4 = consts.tile([1, 4], F32)
        nc.scalar.activation(bet4, abr[:, 4:8], ACT.Sigmoid)
        # prewarm the Silu table set now (Square/Copy live in it too, so
        # no further ACT table load happens for the rest of the kernel)
        prew = consts.tile([1, 1], F32)
        nc.vector.memset(prew, 1.0)
        nc.scalar.activation(prew, prew, ACT.Silu)
        # broadcast g_t down all 128 partitions (ones-column matmul)
        gtb_ps = psB.tile([128, 4], F32, tag="tp", padded_shape=[128, 512])
        nc.tensor.matmul(gtb_ps, ones_r, gt4, start=True, stop=True)
        gtb = consts.tile([128, 4], F32)
        nc.vector.tensor_copy(gtb, gtb_ps)

        # ---- W_z first half on PE (slow fp32 path gets the whole
        # kernel duration: its chunks are DMA'd before everything else)
        mz = psA.tile([1, 512], F32, tag="a")
        for j in range(8):
            wt = wpool.tile([128, 512], F32, tag="wz", name=f"wzp_{j}", bufs=8)
            nc.sync.dma_start(out=wt, in_=wzp_t[j])
            nc.tensor.matmul(
                mz, x_sb[:, j : j + 1], wt, start=(j == 0), stop=(j == 7)
            )
        # W_z second half (x 1024..2047) on DVE+ACT, natural layout
        zac2 = consts.tile([128, 4], F32)
        for t in range(4):
            wt = wpool.tile([128, 1024], F32, tag="wzn", name=f"wzn_{t}", bufs=4)
            nc.sync.dma_start(out=wt, in_=wzn_t[t])
            sc = wpool.tile([128, 1024], F32, tag="scrz", name=f"zscr_{t}", bufs=2)
            nc.vector.tensor_mul(sc, wt, x_b[:, 1024:2048])
            nc.scalar.activation(sc, sc, ACT.Copy, accum_out=zac2[:, t : t + 1])

        # ---- W_qkv mat-vec on DVE+ACT (natural layout, column accum) ---
        # tiles 0..3 are the q/k channels: everything that depends only on
        # q/k (conv half, silu, norms, k^T S mat-muls, transposes) runs
        # while tiles 4..7 (v) and W_out are still streaming.
        macc = consts.tile([128, 8], F32)

        def wq_tile(t):
            wt = wpool.tile([128, H], F32, tag="w", name=f"wq_{t}")
            nc.sync.dma_start(out=wt, in_=wq_t[t])
            sc = wpool.tile([128, H], F32, tag="scr", name=f"qscr_{t}", bufs=3)
            nc.vector.tensor_mul(sc, wt, x_b)
            nc.scalar.activation(sc, sc, ACT.Copy, accum_out=macc[:, t : t + 1])

        for t in range(4):
            wq_tile(t)

        # depthwise conv (k=4) + silu in column form, per 4-column half
        prod = consts.tile([128, 8, 4], F32)
        cacc = consts.tile([128, 8], F32)
        ma_s = consts.tile([128, 8], F32)

        def conv_half(h):
            sl = slice(4 * h, 4 * h + 4)
            nc.vector.tensor_copy(cmb[:, sl, 3], macc[:, sl])
            nc.vector.tensor_mul(prod[:, sl, :], cmb[:, sl, :], cwt[:, sl, :])
            nc.vector.reduce_sum(out=cacc[:, sl], in_=prod[:, sl, :], axis=AX.X)
            nc.scalar.activation(ma_s[:, sl], cacc[:, sl], ACT.Silu)
            nc.scalar.dma_start(out=nv_t[:, sl, :], in_=cmb[:, sl, 1:4])

        conv_half(0)

        def col_to_row(col_ap, tag):
            tp = psB.tile([1, 128], F32, tag="tp", padded_shape=[1, 512],
                          name=f"tpr_{tag}")
            nc.tensor.transpose(tp, col_ap, ident)
            row = consts.tile([1, 128], F32, name=f"row_{tag}", tag=tag)
            nc.vector.tensor_copy(row, tp)
            return row

        def row_to_col(row_ap, tag):
            tp = psB.tile([128, 1], F32, tag="tp", padded_shape=[128, 512],
                          name=f"tpc_{tag}")
            nc.tensor.transpose(tp, row_ap, ident[0:1, 0:1])
            col = consts.tile([128, 1], F32, name=f"col_{tag}", tag=tag)
            nc.vector.tensor_copy(col, tp)
            return col

        # k rows for the outer products (q/k half is ready)
        krow = [col_to_row(ma_s[:, 2 + g : 3 + g], f"kr{g}") for g in range(2)]

        # q/k L2 norms: PE Gram mat-muls
        sqr = consts.tile([1, 4], F32)
        for j in range(4):  # columns q0, q1, k0, k1
            sq_ps = psC.tile([1, 1], F32, tag="rps", name=f"sq_{j}",
                             padded_shape=[1, 128])
            nc.tensor.matmul(
                sq_ps, ma_s[:, j : j + 1], ma_s[:, j : j + 1], start=True, stop=True
            )
            nc.vector.tensor_copy(sqr[:, j : j + 1], sq_ps)

        for t in range(4, 8):
            wq_tile(t)
        conv_half(1)
        # z = mz (PE half, psum row) + zac2 (DVE half, columns)
        zfull = consts.tile([1, 512], F32)
        nc.vector.tensor_copy(zfull, mz)
        zs = consts.tile([1, 512], F32)
        for i in range(4):
            zr2 = col_to_row(zac2[:, i : i + 1], f"z2_{i}")
            nc.vector.tensor_add(
                zfull[:, i * 128 : (i + 1) * 128],
                zfull[:, i * 128 : (i + 1) * 128], zr2,
            )
        nc.scalar.activation(zs, zfull, ACT.Silu)
        # v rows (per head)
        vrow = [col_to_row(ma_s[:, 4 + i : 5 + i], f"vr{i}") for i in range(4)]

        # W_out PE quarter (h 1536..2047, transposed): DMA'd before wo1
        wo2_sb = consts.tile([128, 4, 512], F32)
        for j in range(4):
            nc.scalar.dma_start(out=wo2_sb[:, j, :], in_=wo2_t[j])
        # W_out h rows 0..1535, natural layout, preloaded for DVE+ACT
        wo1_sb = consts.tile([128, 12, 512], F32)
        for hf in range(3):
            nc.scalar.dma_start(
                out=wo1_sb[:, hf * 4 : (hf + 1) * 4, :],
                in_=wo1_t[:, hf * 4 : (hf + 1) * 4, :],
            )

        magic4 = consts.tile([1, 4], I32)
        nc.vector.memset(magic4, MAGIC)

        def newton_rsqrt(out, v_ap, pref):
            """out = 1/sqrt(v) on DVE only (quake seed + 3 Newton steps)."""
            sh = list(v_ap.shape)
            tsh = consts.tile(sh, I32, name=f"{pref}_i")
            nc.vector.tensor_scalar(
                out=tsh, in0=v_ap.bitcast(I32), scalar1=1, scalar2=None,
                op0=OP.logical_shift_right,
            )
            nc.vector.tensor_sub(out.bitcast(I32), magic4[:, 0 : sh[1]], tsh)
            hv = consts.tile(sh, F32, name=f"{pref}_hv")
            nc.vector.tensor_scalar_mul(hv, v_ap, 0.5)
            aa = consts.tile(sh, F32, name=f"{pref}_a")
            for _ in range(2):
                nc.vector.tensor_mul(aa, out, out)
                nc.vector.tensor_mul(aa, aa, hv)
                nc.vector.tensor_scalar(
                    out=aa, in0=aa, scalar1=-1.0, scalar2=1.5, op0=OP.mult, op1=OP.add
                )
                nc.vector.tensor_mul(out, out, aa)

        rinv = consts.tile([1, 4], F32)
        sqe = consts.tile([1, 4], F32)
        nc.vector.tensor_scalar_add(sqe, sqr, EPS)
        newton_rsqrt(rinv, sqe, "ri")
        # per-head (x4) expansions: head i uses q/k norm i//2
        rqh = consts.tile([1, 4], F32)
        nc.vector.tensor_copy(rqh.rearrange("a (b c) -> a b c", c=2), _rep2(rinv[:, 0:2]))
        rkh = consts.tile([1, 4], F32)
        nc.vector.tensor_copy(rkh.rearrange("a (b c) -> a b c", c=2), _rep2(rinv[:, 2:4]))
        # gkn = -g_t * rinv_k ; bk = beta * rinv_k ; qs = rinv_q / sqrt(128)
        gkn = consts.tile([1, 4], F32)
        nc.vector.tensor_mul(gkn, gt4, rkh)
        nc.vector.tensor_scalar_mul(gkn, gkn, -1.0)
        bk4 = consts.tile([1, 4], F32)
        nc.vector.tensor_mul(bk4, bet4, rkh)
        qs4 = consts.tile([1, 4], F32)
        nc.vector.tensor_scalar_mul(qs4, rqh, QSCALE)

        # ---- delta rule, batched across the 4 heads so PE / DVE / ACT
        # pipeline instead of serializing one head at a time -------------
        s_new = consts.tile([128, 4, 128], F32)
        g1 = [
            consts.tile([1, 128], F32, name=f"g1_{i}", tag=f"g1{i}") for i in range(4)
        ]
        ssr = consts.tile([1, 4], F32)
        kv_ps = [psC.tile([1, 128], F32, tag="rps", name=f"kv_{i}") for i in range(4)]
        for i in range(4):
            nc.tensor.matmul(
                kv_ps[i], ma_s[:, 2 + i // 2 : 3 + i // 2], s_all[:, i, :],
                start=True, stop=True,
            )
        dk = [rowp.tile([1, 128], F32, tag=f"dk{i}", name=f"dk_{i}") for i in range(4)]
        for i in range(4):
            # dk = (v - g_t*rinv_k*(k^T S)) * beta * rinv_k
            dkp = rowp.tile([1, 128], F32, tag="dkp", name=f"dkp_{i}", bufs=2)
            nc.vector.scalar_tensor_tensor(
                out=dkp, in0=kv_ps[i], scalar=gkn[:, i : i + 1],
                in1=vrow[i], op0=OP.mult, op1=OP.add,
            )
            nc.vector.tensor_scalar_mul(dk[i], dkp, bk4[:, i : i + 1])
        for i in range(4):
            out_ps = psB.tile([128, 128], F32, tag="tp", padded_shape=[128, 512],
                              name=f"outer_{i}")
            nc.tensor.matmul(out_ps, krow[i // 2], dk[i], start=True, stop=True)
            # s_new = s * g_t + k (x) dk
            nc.vector.scalar_tensor_tensor(
                out=s_new[:, i, :], in0=s_all[:, i, :], scalar=gtb[:, i : i + 1],
                in1=out_ps, op0=OP.mult, op1=OP.add,
            )
        o_ps = [psC.tile([1, 128], F32, tag="rps", name=f"o_{i}") for i in range(4)]
        for i in range(4):
            nc.sync.dma_start(out=ns_t[i], in_=s_new[:, i, :])
            nc.tensor.matmul(
                o_ps[i], ma_s[:, i // 2 : i // 2 + 1], s_new[:, i, :],
                start=True, stop=True,
            )
        for i in range(4):
            # g1 = (q^T S_new) * (rinv_q/sqrt(128)) * silu(z)
            nc.vector.scalar_tensor_tensor(
                out=g1[i], in0=o_ps[i], scalar=qs4[:, i : i + 1],
                in1=zs[:, i * 128 : (i + 1) * 128], op0=OP.mult, op1=OP.mult,
            )
        for i in range(4):
            rsc = rowp.tile([1, 128], F32, tag="rsc", name=f"rsc_{i}", bufs=2)
            nc.scalar.activation(rsc, g1[i], ACT.Square, accum_out=ssr[:, i : i + 1])

        # ---- gated RMS norm scales -------------------------------------
        rr4 = consts.tile([1, 4], F32)
        nc.vector.tensor_scalar(
            out=rr4, in0=ssr, scalar1=1.0 / 128.0, scalar2=EPS,
            op0=OP.mult, op1=OP.add,
        )
        rrs = consts.tile([1, 4], F32)
        newton_rsqrt(rrs, rr4, "rr")
        y_row = consts.tile([1, 512], F32)
        for i in range(4):
            nc.vector.scalar_tensor_tensor(
                out=y_row[:, i * 128 : (i + 1) * 128], in0=g1[i],
                scalar=rrs[:, i : i + 1], in1=nwr, op0=OP.mult, op1=OP.mult,
            )

        # ---- out projection: half on DVE+ACT, half on PE ---------------
        # y broadcast for the DVE half
        yb_ps = psB.tile([128, 512], F32, tag="yb", bufs=1)
        yb = consts.tile([128, 512], F32)
        for i in range(4):
            nc.tensor.matmul(
                yb_ps[:, i * 128 : (i + 1) * 128], ones_r,
                y_row[:, i * 128 : (i + 1) * 128], start=True, stop=True,
            )
            nc.vector.tensor_copy(
                yb[:, i * 128 : (i + 1) * 128], yb_ps[:, i * 128 : (i + 1) * 128]
            )
        oacc = consts.tile([128, 12], F32)
        for t in range(12):
            sc = wpool.tile([128, 512], F32, tag="scr2", name=f"oscr_{t}", bufs=2)
            nc.vector.tensor_mul(sc, wo1_sb[:, t, :], yb)
            nc.scalar.activation(sc, sc, ACT.Copy, accum_out=oacc[:, t : t + 1])
        nc.scalar.dma_start(out=o1_t[:], in_=oacc)
        # PE quarter: y columns stationary, preloaded W_out^T chunks
        ycol = [row_to_col(y_row[:, j * 128 : (j + 1) * 128], f"yc{j}") for j in range(4)]
        w2 = psA.tile([1, 512], F32, tag="b", name="wacc2")
        for j in range(4):
            nc.tensor.matmul(
                w2, ycol[j], wo2_sb[:, j, :], start=(j == 0), stop=(j == 3)
            )
        op2 = consts.tile([1, 512], F32)
        nc.vector.tensor_copy(op2, w2)
        nc.scalar.dma_start(out=o2_t[:], in_=op2)

    nc.compile()
    return nc


_NC = None


def _get_nc():
    global _NC
    if _NC is None:
        _NC = build_nc()
    return _NC


def _core_channels(c):
    return np.concatenate(
        [
            np.arange(256) + 2 * c * 128,          # q channels
            np.arange(256) + 2048 + 2 * c * 128,   # k channels
            np.arange(512) + 4096 + 4 * c * 128,   # v channels
        ]
    )


def shard_inputs(x, state, conv_state, W_qkv, W_z, W_b, W_a, conv_w, A_log,
                 dt_bias, norm_w, W_out):
    x = np.ascontiguousarray(np.asarray(x, np.float32).reshape(16, 128))
    state = np.asarray(state, np.float32).reshape(32, 128, 128)
    conv_state = np.asarray(conv_state, np.float32).reshape(8192, 3)
    W_qkv = np.asarray(W_qkv, np.float32)
    W_z = np.asarray(W_z, np.float32)
    W_b = np.asarray(W_b, np.float32)
    W_a = np.asarray(W_a, np.float32)
    conv_w = np.asarray(conv_w, np.float32).reshape(8192, 4)
    A_log = np.asarray(A_log, np.float32)
    dt_bias = np.asarray(dt_bias, np.float32)
    norm_w = np.asarray(norm_w, np.float32)
    W_out = np.asarray(W_out, np.float32)

    in_maps = []
    for c in range(N_CORES):
        hs = slice(4 * c, 4 * c + 4)
        chs = _core_channels(c)
        wq = np.ascontiguousarray(W_qkv[chs].reshape(8, 128, H))
        wz_c = W_z[4 * c * 128 : (4 * c + 4) * 128]       # [512, 2048]
        wzT = np.ascontiguousarray(wz_c[:, 0:1024].T).reshape(8, 128, 512)
        wzn = np.ascontiguousarray(wz_c[:, 1024:2048].reshape(4, 128, 1024))
        wab = np.concatenate([W_a[hs], W_b[hs]], axis=0)  # [8, 2048]
        wabT = np.ascontiguousarray(
            wab.T.reshape(16, 128, 8).transpose(1, 0, 2)
        )  # [128, 16, 8]
        wo_c = W_out[:, 512 * c : 512 * (c + 1)]          # [2048, 512]
        wo1 = np.ascontiguousarray(
            wo_c[0:1536].reshape(12, 128, 512).transpose(1, 0, 2)
        )  # [128, 12, 512]
        wo2T = np.ascontiguousarray(wo_c[1536:2048].T).reshape(4, 128, 512)
        cs = np.ascontiguousarray(conv_state[chs].reshape(8, 128, 3).transpose(1, 0, 2))
        cw = np.ascontiguousarray(conv_w[chs].reshape(8, 128, 4).transpose(1, 0, 2))
        st = np.ascontiguousarray(state[hs])
        in_maps.append(
            dict(
                x=x, wq=wq, wzT=wzT, wzn=wzn, wabT=wabT, wo1=wo1, wo2T=wo2T,
                cs=cs, cw=cw, st=st,
                ident=np.eye(128, dtype=np.float32),
                onesr=np.ones((1, 128), np.float32),
                alog=np.ascontiguousarray(A_log[hs].reshape(1, 4)),
                dtb=np.ascontiguousarray(dt_bias[hs].reshape(1, 4)),
                nw=np.ascontiguousarray(norm_w.reshape(1, 128)),
            )
        )
    return in_maps


def combine_outputs(results):
    out = np.zeros(H, np.float32)
    new_state = np.zeros((32, 128, 128), np.float32)
    new_conv = np.zeros((8192, 3), np.float32)
    for c in range(N_CORES):
        r = results[c]
        out[0:1536] += r["out_p1"].T.reshape(1536)
        out[1536:2048] += r["out_p2"].reshape(512)
        new_state[4 * c : 4 * c + 4] = r["nst"]
        new_conv[_core_channels(c)] = r["ncv"].transpose(1, 0, 2).reshape(1024, 3)
    return (
        out.reshape(1, 1, H),
        new_state.reshape(1, 32, 128, 128),
        new_conv.reshape(1, 8192, 3),
    )


def run_sharded(inputs, **run_kwargs):
    nc = _get_nc()
    in_maps = shard_inputs(**inputs)
    res = run_bass_kernel_spmd(nc, in_maps, list(range(N_CORES)), **run_kwargs)
    return combine_outputs(res.results), res


def kernel(**inputs):
    outs, _ = run_sharded(inputs)
    return outs


# revision 25
# speedup vs baseline: 1.0583x; 1.0583x over previous
"""DeltaNet single-token decode step on 8 Trainium2 NeuronCores.

Sharding: tensor-parallel over the 32 delta-rule heads. Core c owns heads
4c..4c+3 (which share q/k heads 2c and 2c+1), the matching 1024 rows of
W_qkv, 512 rows of W_z, the matching 1024 depthwise-conv channels, the
4 state slabs, and columns 512c..512c+512 of W_out. Each core produces a
partial out-projection (summed on host), its new-state shard and its
new-conv shard. No device collectives are needed.

The kernel is memory bound: ~16.5 MiB of weights stream through each
core exactly once (~48 us of DMA at the ~360 GB/s per-core HBM rate), so
the mat-vec work is split across three engines to keep each under that
floor (fp32 PE matmul streams at ~1/4 the bf16 rate, so PE alone cannot):
  - W_qkv (8 MiB) runs as DVE multiply + ACT Copy-accumulate (the
    working fused-reduce on this runtime) against a partition-broadcast
    x; mixed lands as per-channel columns, which makes the depthwise
    conv and silu cheap [128,8] column ops.
  - W_z (2 MiB) and half of W_out run on the tensor engine with
    host-transposed chunks and x / y as stationary [128,1] columns.
  - The other half of W_out runs on DVE+ACT, so the final reduction
    tail is split across engines.
  - rsqrt is a DVE Newton iteration (quake seed; no ACT tables, no slow
    DVE reciprocal). Sums of squares use PE Gram mat-muls / ACT
    Square-accumulate. Square/Copy live in every ACT table set and the
    real transcendentals (Exp/Ln/Sigmoid/Silu) are all first issued
    early (the a/b projection uses its own tiny weights, done in the
    first microseconds), so no table-set load lands on the critical path.
"""

import numpy as np

import concourse.bacc as bacc
import concourse.bass as bass
import concourse.tile as tile
from concourse import mybir
from concourse.bass_utils import run_bass_kernel_spmd

F32 = mybir.dt.float32
I32 = mybir.dt.int32
AX = mybir.AxisListType
OP = mybir.AluOpType
ACT = mybir.ActivationFunctionType

N_CORES = 8
H = 2048
EPS = 1e-6
QSCALE = float(1.0 / np.sqrt(128.0))
MAGIC = 0x5F3759DF


def _rep2(ap):
    """View a [1, n] AP as [1, n, 2] reading each element twice (free step 0)."""
    return bass.AP(tensor=ap.tensor, offset=ap.offset, ap=list(ap.ap) + [[0, 2]])


def _bcast_part(ap, p):
    """View an AP as having p partitions with partition step 0 (for DMA)."""
    return bass.AP(tensor=ap.tensor, offset=ap.offset, ap=[[0, p]] + list(ap.ap))


def build_nc():
    nc = bacc.Bacc("TRN2", target_bir_lowering=False, debug=False)

    # inputs (per-core shards; layouts prearranged on host)
    x_t = nc.dram_tensor("x", [16, 128], F32, kind="ExternalInput")
    wq_t = nc.dram_tensor("wq", [8, 128, H], F32, kind="ExternalInput")
    wzp_t = nc.dram_tensor("wzT", [8, 128, 512], F32, kind="ExternalInput")
    wzn_t = nc.dram_tensor("wzn", [4, 128, 1024], F32, kind="ExternalInput")
    wab_t = nc.dram_tensor("wabT", [128, 16, 8], F32, kind="ExternalInput")
    wo1_t = nc.dram_tensor("wo1", [128, 12, 512], F32, kind="ExternalInput")
    wo2_t = nc.dram_tensor("wo2T", [4, 128, 512], F32, kind="ExternalInput")
    cs_t = nc.dram_tensor("cs", [128, 8, 3], F32, kind="ExternalInput")
    cw_t = nc.dram_tensor("cw", [128, 8, 4], F32, kind="ExternalInput")
    st_t = nc.dram_tensor("st", [4, 128, 128], F32, kind="ExternalInput")
    al_t = nc.dram_tensor("alog", [1, 4], F32, kind="ExternalInput")
    db_t = nc.dram_tensor("dtb", [1, 4], F32, kind="ExternalInput")
    nw_t = nc.dram_tensor("nw", [1, 128], F32, kind="ExternalInput")
    id_t = nc.dram_tensor("ident", [128, 128], F32, kind="ExternalInput")
    on_t = nc.dram_tensor("onesr", [1, 128], F32, kind="ExternalInput")

    # outputs
    o1_t = nc.dram_tensor("out_p1", [128, 12], F32, kind="ExternalOutput")
    o2_t = nc.dram_tensor("out_p2", [1, 512], F32, kind="ExternalOutput")
    ns_t = nc.dram_tensor("nst", [4, 128, 128], F32, kind="ExternalOutput")
    nv_t = nc.dram_tensor("ncv", [128, 8, 3], F32, kind="ExternalOutput")

    with (
        tile.TileContext(nc) as tc,
        tc.tile_pool(name="consts", bufs=1) as consts,
        tc.tile_pool(name="wpool", bufs=4) as wpool,
        tc.tile_pool(name="rows", bufs=2) as rowp,
        tc.tile_pool(name="psA", bufs=1, space="PSUM") as psA,
        tc.tile_pool(name="psB", bufs=2, space="PSUM") as psB,
        tc.tile_pool(name="psC", bufs=3, space="PSUM") as psC,
    ):
        # ---- constants / setup ----------------------------------------
        # x broadcast first in the SWDGE queue: the whole W_qkv DVE
        # stream gates on it, and each earlier SWDGE op costs ~2 us
        x_b = consts.tile([128, H], F32)
        nc.gpsimd.dma_start(
            out=x_b, in_=_bcast_part(x_t[:].rearrange("a b -> (a b)"), 128)
        )
        ident = consts.tile([128, 128], F32)
        nc.gpsimd.dma_start(out=ident, in_=id_t[:])
        ones_r = consts.tile([1, 128], F32)
        nc.gpsimd.dma_start(out=ones_r, in_=on_t[:])

        # x: [16,128] rows -> one transpose -> [128,16] stationary columns
        x16 = consts.tile([16, 128], F32)
        nc.sync.dma_start(out=x16, in_=x_t[:])
        xt_ps = psB.tile([128, 16], F32, tag="tp", padded_shape=[128, 512])
        nc.tensor.transpose(xt_ps, x16, ident[0:16, 0:16])
        x_sb = consts.tile([128, 16], F32)
        nc.vector.tensor_copy(x_sb, xt_ps)

        # small loads
        alr = consts.tile([1, 4], F32)
        nc.gpsimd.dma_start(out=alr, in_=al_t[:])
        dbr = consts.tile([1, 4], F32)
        nc.gpsimd.dma_start(out=dbr, in_=db_t[:])
        nwr = consts.tile([1, 128], F32)
        nc.gpsimd.dma_start(out=nwr, in_=nw_t[:])
        cmb = consts.tile([128, 8, 4], F32)
        nc.gpsimd.dma_start(out=cmb[:, :, 0:3], in_=cs_t[:])
        cwt = consts.tile([128, 8, 4], F32)
        nc.gpsimd.dma_start(out=cwt, in_=cw_t[:])
        s_all = consts.tile([128, 4, 128], F32)
        nc.gpsimd.dma_start(out=s_all, in_=st_t[:].rearrange("i p v -> p i v"))
        wab_sb = consts.tile([128, 16, 8], F32)
        nc.gpsimd.dma_start(out=wab_sb, in_=wab_t[:])

        # ---- a/b projection first (tiny weights): scalars come early ---
        mab = psA.tile([1, 8], F32, tag="b", padded_shape=[1, 512])
        for j in range(16):
            nc.tensor.matmul(
                mab, x_sb[:, j : j + 1], wab_sb[:, j, :],
                start=(j == 0), stop=(j == 15),
            )
        abr = consts.tile([1, 8], F32)
        nc.vector.tensor_copy(abr, mab)
        # softplus(a + dt_bias) = ln(1 + exp(.)) ; exps batched before ln
        sp4 = consts.tile([1, 4], F32)
        nc.vector.tensor_add(sp4, abr[:, 0:4], dbr)
        nc.scalar.activation(sp4, sp4, ACT.Exp)
        ea4 = consts.tile([1, 4], F32)
        nc.scalar.activation(ea4, alr, ACT.Exp)
        nc.vector.tensor_scalar_add(sp4, sp4, 1.0)
        nc.scalar.activation(sp4, sp4, ACT.Ln)
        # g_t = exp(-exp(A_log) * softplus)
        gt4 = consts.tile([1, 4], F32)
        nc.vector.tensor_mul(gt4, ea4, sp4)
        nc.scalar.activation(gt4, gt4, ACT.Exp, scale=-1.0)
        # beta = sigmoid(b)
        bet4 = consts.tile([1, 4], F32)
        nc.scalar.activation(bet4, abr[:, 4:8], ACT.Sigmoid)
        # prewarm the Silu table set now (Square/Copy live in it too, so
        # no further ACT table load happens for the rest of the kernel)
        prew = consts.tile([1, 1], F32)
        nc.vector.memset(prew, 1.0)
        nc.scalar.activation(prew, prew, ACT.Silu)
        # broadcast g_t down all 128 partitions (ones-column matmul)
        gtb_ps = psB.tile([128, 4], F32, tag="tp", padded_shape=[128, 512])
        nc.tensor.matmul(gtb_ps, ones_r, gt4, start=True, stop=True)
        gtb = consts.tile([128, 4], F32)
        nc.vector.tensor_copy(gtb, gtb_ps)

        # ---- W_z first half on PE (slow fp32 path gets the whole
        # kernel duration: its chunks are DMA'd before everything else)
        mz = psA.tile([1, 512], F32, tag="a")
        for j in range(8):
            wt = wpool.tile([128, 512], F32, tag="wz", name=f"wzp_{j}", bufs=8)
            nc.sync.dma_start(out=wt, in_=wzp_t[j])
            nc.tensor.matmul(
                mz, x_sb[:, j : j + 1], wt, start=(j == 0), stop=(j == 7)
            )
        # W_z second half (x 1024..2047) on DVE+ACT, natural layout
        zac2 = consts.tile([128, 4], F32)
        for t in range(4):
            wt = wpool.tile([128, 1024], F32, tag="wzn", name=f"wzn_{t}", bufs=4)
            nc.sync.dma_start(out=wt, in_=wzn_t[t])
            sc = wpool.tile([128, 1024], F32, tag="scrz", name=f"zscr_{t}", bufs=2)
            nc.vector.tensor_mul(sc, wt, x_b[:, 1024:2048])
            nc.scalar.activation(sc, sc, ACT.Copy, accum_out=zac2[:, t : t + 1])

        # ---- W_qkv mat-vec on DVE+ACT (natural layout, column accum) ---
        # tiles 0..3 are the q/k channels: everything that depends only on
        # q/k (conv half, silu, norms, k^T S mat-muls, transposes) runs
        # while tiles 4..7 (v) and W_out are still streaming.
        macc = consts.tile([128, 8], F32)

        def wq_tile(t):
            wt = wpool.tile([128, H], F32, tag="w", name=f"wq_{t}")
            nc.sync.dma_start(out=wt, in_=wq_t[t])
            sc = wpool.tile([128, H], F32, tag="scr", name=f"qscr_{t}", bufs=3)
            nc.vector.tensor_mul(sc, wt, x_b)
            nc.scalar.activation(sc, sc, ACT.Copy, accum_out=macc[:, t : t + 1])

        for t in range(4):
            wq_tile(t)

        # depthwise conv (k=4) + silu in column form, per 4-column half
        prod = consts.tile([128, 8, 4], F32)
        cacc = consts.tile([128, 8], F32)
        ma_s = consts.tile([128, 8], F32)

        def conv_half(h):
            sl = slice(4 * h, 4 * h + 4)
            nc.vector.tensor_copy(cmb[:, sl, 3], macc[:, sl])
            nc.vector.tensor_mul(prod[:, sl, :], cmb[:, sl, :], cwt[:, sl, :])
            nc.vector.reduce_sum(out=cacc[:, sl], in_=prod[:, sl, :], axis=AX.X)
            nc.scalar.activation(ma_s[:, sl], cacc[:, sl], ACT.Silu)
            nc.scalar.dma_start(out=nv_t[:, sl, :], in_=cmb[:, sl, 1:4])

        conv_half(0)

        def col_to_row(col_ap, tag):
            tp = psB.tile([1, 128], F32, tag="tp", padded_shape=[1, 512],
                          name=f"tpr_{tag}")
            nc.tensor.transpose(tp, col_ap, ident)
            row = consts.tile([1, 128], F32, name=f"row_{tag}", tag=tag)
            nc.vector.tensor_copy(row, tp)
            return row

        def row_to_col(row_ap, tag):
            tp = psB.tile([128, 1], F32, tag="tp", padded_shape=[128, 512],
                          name=f"tpc_{tag}")
            nc.tensor.transpose(tp, row_ap, ident[0:1, 0:1])
            col = consts.tile([128, 1], F32, name=f"col_{tag}", tag=tag)
            nc.vector.tensor_copy(col, tp)
            return col

        # k rows for the outer products (q/k half is ready)
        krow = [col_to_row(ma_s[:, 2 + g : 3 + g], f"kr{g}") for g in range(2)]

        # q/k L2 norms: PE Gram mat-muls
        sqr = consts.tile([1, 4], F32)
        for j in range(4):  # columns q0, q1, k0, k1
            sq_ps = psC.tile([1, 1], F32, tag="rps", name=f"sq_{j}",
                             padded_shape=[1, 128])
            nc.tensor.matmul(
                sq_ps, ma_s[:, j : j + 1], ma_s[:, j : j + 1], start=True, stop=True
            )
            nc.vector.tensor_copy(sqr[:, j : j + 1], sq_ps)

        for t in range(4, 8):
            wq_tile(t)
        conv_half(1)
        # z = mz (PE half, psum row) + zac2 (DVE half, columns)
        zfull = consts.tile([1, 512], F32)
        nc.vector.tensor_copy(zfull, mz)
        zs = consts.tile([1, 512], F32)
        for i in range(4):
            zr2 = col_to_row(zac2[:, i : i + 1], f"z2_{i}")
            nc.vector.tensor_add(
                zfull[:, i * 128 : (i + 1) * 128],
                zfull[:, i * 128 : (i + 1) * 128], zr2,
            )
        nc.scalar.activation(zs, zfull, ACT.Silu)
        # v rows (per head)
        vrow = [col_to_row(ma_s[:, 4 + i : 5 + i], f"vr{i}") for i in range(4)]

        # W_out PE quarter (h 1536..2047, transposed): DMA'd before wo1
        wo2_sb = consts.tile([128, 4, 512], F32)
        for j in range(4):
            nc.sync.dma_start(out=wo2_sb[:, j, :], in_=wo2_t[j])
        # W_out h rows 0..1535, natural layout, preloaded for DVE+ACT
        wo1_sb = consts.tile([128, 12, 512], F32)
        for hf in range(3):
            nc.sync.dma_start(
                out=wo1_sb[:, hf * 4 : (hf + 1) * 4, :],
                in_=wo1_t[:, hf * 4 : (hf + 1) * 4, :],
            )

        magic4 = consts.tile([1, 4], I32)
        nc.vector.memset(magic4, MAGIC)

        def newton_rsqrt(out, v_ap, pref):
            """out = 1/sqrt(v) on DVE only (quake seed + 3 Newton steps)."""
            sh = list(v_ap.shape)
            tsh = consts.tile(sh, I32, name=f"{pref}_i")
            nc.vector.tensor_scalar(
                out=tsh, in0=v_ap.bitcast(I32), scalar1=1, scalar2=None,
                op0=OP.logical_shift_right,
            )
            nc.vector.tensor_sub(out.bitcast(I32), magic4[:, 0 : sh[1]], tsh)
            hv = consts.tile(sh, F32, name=f"{pref}_hv")
            nc.vector.tensor_scalar_mul(hv, v_ap, 0.5)
            aa = consts.tile(sh, F32, name=f"{pref}_a")
            for _ in range(2):
                nc.vector.tensor_mul(aa, out, out)
                nc.vector.tensor_mul(aa, aa, hv)
                nc.vector.tensor_scalar(
                    out=aa, in0=aa, scalar1=-1.0, scalar2=1.5, op0=OP.mult, op1=OP.add
                )
                nc.vector.tensor_mul(out, out, aa)

        rinv = consts.tile([1, 4], F32)
        sqe = consts.tile([1, 4], F32)
        nc.vector.tensor_scalar_add(sqe, sqr, EPS)
        newton_rsqrt(rinv, sqe, "ri")
        # per-head (x4) expansions: head i uses q/k norm i//2
        rqh = consts.tile([1, 4], F32)
        nc.vector.tensor_copy(rqh.rearrange("a (b c) -> a b c", c=2), _rep2(rinv[:, 0:2]))
        rkh = consts.tile([1, 4], F32)
        nc.vector.tensor_copy(rkh.rearrange("a (b c) -> a b c", c=2), _rep2(rinv[:, 2:4]))
        # gkn = -g_t * rinv_k ; bk = beta * rinv_k ; qs = rinv_q / sqrt(128)
        gkn = consts.tile([1, 4], F32)
        nc.vector.tensor_mul(gkn, gt4, rkh)
        nc.vector.tensor_scalar_mul(gkn, gkn, -1.0)
        bk4 = consts.tile([1, 4], F32)
        nc.vector.tensor_mul(bk4, bet4, rkh)
        qs4 = consts.tile([1, 4], F32)
        nc.vector.tensor_scalar_mul(qs4, rqh, QSCALE)

        # ---- delta rule, batched across the 4 heads so PE / DVE / ACT
        # pipeline instead of serializing one head at a time -------------
        s_new = consts.tile([128, 4, 128], F32)
        g1 = [
            consts.tile([1, 128], F32, name=f"g1_{i}", tag=f"g1{i}") for i in range(4)
        ]
        ssr = consts.tile([1, 4], F32)
        kv_ps = [psC.tile([1, 128], F32, tag="rps", name=f"kv_{i}") for i in range(4)]
        for i in range(4):
            nc.tensor.matmul(
                kv_ps[i], ma_s[:, 2 + i // 2 : 3 + i // 2], s_all[:, i, :],
                start=True, stop=True,
            )
        dk = [rowp.tile([1, 128], F32, tag=f"dk{i}", name=f"dk_{i}") for i in range(4)]
        for i in range(4):
            # dk = (v - g_t*rinv_k*(k^T S)) * beta * rinv_k
            dkp = rowp.tile([1, 128], F32, tag="dkp", name=f"dkp_{i}", bufs=2)
            nc.vector.scalar_tensor_tensor(
                out=dkp, in0=kv_ps[i], scalar=gkn[:, i : i + 1],
                in1=vrow[i], op0=OP.mult, op1=OP.add,
            )
            nc.vector.tensor_scalar_mul(dk[i], dkp, bk4[:, i : i + 1])
        for i in range(4):
            out_ps = psB.tile([128, 128], F32, tag="tp", padded_shape=[128, 512],
                              name=f"outer_{i}")
            nc.tensor.matmul(out_ps, krow[i // 2], dk[i], start=True, stop=True)
            # s_new = s * g_t + k (x) dk
            nc.vector.scalar_tensor_tensor(
                out=s_new[:, i, :], in0=s_all[:, i, :], scalar=gtb[:, i : i + 1],
                in1=out_ps, op0=OP.mult, op1=OP.add,
            )
        o_ps = [psC.tile([1, 128], F32, tag="rps", name=f"o_{i}") for i in range(4)]
        for i in range(4):
            nc.sync.dma_start(out=ns_t[i], in_=s_new[:, i, :])
            nc.tensor.matmul(
                o_ps[i], ma_s[:, i // 2 : i // 2 + 1], s_new[:, i, :],
                start=True, stop=True,
            )
        for i in range(4):
            # g1 = (q^T S_new) * (rinv_q/sqrt(128)) * silu(z)
            nc.vector.scalar_tensor_tensor(
                out=g1[i], in0=o_ps[i], scalar=qs4[:, i : i + 1],
                in1=zs[:, i * 128 : (i + 1) * 128], op0=OP.mult, op1=OP.mult,
            )
        for i in range(4):
            rsc = rowp.tile([1, 128], F32, tag="rsc", name=f"rsc_{i}", bufs=2)
            nc.scalar.activation(rsc, g1[i], ACT.Square, accum_out=ssr[:, i : i + 1])

        # ---- gated RMS norm scales -------------------------------------
        rr4 = consts.tile([1, 4], F32)
        nc.vector.tensor_scalar(
            out=rr4, in0=ssr, scalar1=1.0 / 128.0, scalar2=EPS,
            op0=OP.mult, op1=OP.add,
        )
        rrs = consts.tile([1, 4], F32)
        newton_rsqrt(rrs, rr4, "rr")
        y_row = consts.tile([1, 512], F32)
        for i in range(4):
            nc.vector.scalar_tensor_tensor(
                out=y_row[:, i * 128 : (i + 1) * 128], in0=g1[i],
                scalar=rrs[:, i : i + 1], in1=nwr, op0=OP.mult, op1=OP.mult,
            )

        # ---- out projection: half on DVE+ACT, half on PE ---------------
        # y broadcast for the DVE half
        yb_ps = psB.tile([128, 512], F32, tag="yb", bufs=1)
        yb = consts.tile([128, 512], F32)
        for i in range(4):
            nc.tensor.matmul(
                yb_ps[:, i * 128 : (i + 1) * 128], ones_r,
                y_row[:, i * 128 : (i + 1) * 128], start=True, stop=True,
            )
            nc.vector.tensor_copy(
                yb[:, i * 128 : (i + 1) * 128], yb_ps[:, i * 128 : (i + 1) * 128]
            )
        oacc = consts.tile([128, 12], F32)
        for t in range(12):
            sc = wpool.tile([128, 512], F32, tag="scr2", name=f"oscr_{t}", bufs=2)
            nc.vector.tensor_mul(sc, wo1_sb[:, t, :], yb)
            nc.scalar.activation(sc, sc, ACT.Copy, accum_out=oacc[:, t : t + 1])
        nc.scalar.dma_start(out=o1_t[:], in_=oacc)
        # PE quarter: y columns stationary, preloaded W_out^T chunks
        ycol = [row_to_col(y_row[:, j * 128 : (j + 1) * 128], f"yc{j}") for j in range(4)]
        w2 = psA.tile([1, 512], F32, tag="b", name="wacc2")
        for j in range(4):
            nc.tensor.matmul(
                w2, ycol[j], wo2_sb[:, j, :], start=(j == 0), stop=(j == 3)
            )
        op2 = consts.tile([1, 512], F32)
        nc.vector.tensor_copy(op2, w2)
        nc.scalar.dma_start(out=o2_t[:], in_=op2)

    nc.compile()
    return nc


_NC = None


def _get_nc():
    global _NC
    if _NC is None:
        _NC = build_nc()
    return _NC


def _core_channels(c):
    return np.concatenate(
        [
            np.arange(256) + 2 * c * 128,          # q channels
            np.arange(256) + 2048 + 2 * c * 128,   # k channels
            np.arange(512) + 4096 + 4 * c * 128,   # v channels
        ]
    )


def shard_inputs(x, state, conv_state, W_qkv, W_z, W_b, W_a, conv_w, A_log,
                 dt_bias, norm_w, W_out):
    x = np.ascontiguousarray(np.asarray(x, np.float32).reshape(16, 128))
    state = np.asarray(state, np.float32).reshape(32, 128, 128)
    conv_state = np.asarray(conv_state, np.float32).reshape(8192, 3)
    W_qkv = np.asarray(W_qkv, np.float32)
    W_z = np.asarray(W_z, np.float32)
    W_b = np.asarray(W_b, np.float32)
    W_a = np.asarray(W_a, np.float32)
    conv_w = np.asarray(conv_w, np.float32).reshape(8192, 4)
    A_log = np.asarray(A_log, np.float32)
    dt_bias = np.asarray(dt_bias, np.float32)
    norm_w = np.asarray(norm_w, np.float32)
    W_out = np.asarray(W_out, np.float32)

    in_maps = []
    for c in range(N_CORES):
        hs = slice(4 * c, 4 * c + 4)
        chs = _core_channels(c)
        wq = np.ascontiguousarray(W_qkv[chs].reshape(8, 128, H))
        wz_c = W_z[4 * c * 128 : (4 * c + 4) * 128]       # [512, 2048]
        wzT = np.ascontiguousarray(wz_c[:, 0:1024].T).reshape(8, 128, 512)
        wzn = np.ascontiguousarray(wz_c[:, 1024:2048].reshape(4, 128, 1024))
        wab = np.concatenate([W_a[hs], W_b[hs]], axis=0)  # [8, 2048]
        wabT = np.ascontiguousarray(
            wab.T.reshape(16, 128, 8).transpose(1, 0, 2)
        )  # [128, 16, 8]
        wo_c = W_out[:, 512 * c : 512 * (c + 1)]          # [2048, 512]
        wo1 = np.ascontiguousarray(
            wo_c[0:1536].reshape(12, 128, 512).transpose(1, 0, 2)
        )  # [128, 12, 512]
        wo2T = np.ascontiguousarray(wo_c[1536:2048].T).reshape(4, 128, 512)
        cs = np.ascontiguousarray(conv_state[chs].reshape(8, 128, 3).transpose(1, 0, 2))
        cw = np.ascontiguousarray(conv_w[chs].reshape(8, 128, 4).transpose(1, 0, 2))
        st = np.ascontiguousarray(state[hs])
        in_maps.append(
            dict(
                x=x, wq=wq, wzT=wzT, wzn=wzn, wabT=wabT, wo1=wo1, wo2T=wo2T,
                cs=cs, cw=cw, st=st,
                ident=np.eye(128, dtype=np.float32),
                onesr=np.ones((1, 128), np.float32),
                alog=np.ascontiguousarray(A_log[hs].reshape(1, 4)),
                dtb=np.ascontiguousarray(dt_bias[hs].reshape(1, 4)),
                nw=np.ascontiguousarray(norm_w.reshape(1, 128)),
            )
        )
    return in_maps


def combine_outputs(results):
    out = np.zeros(H, np.float32)
    new_state = np.zeros((32, 128, 128), np.float32)
    new_conv = np.zeros((8192, 3), np.float32)
    for c in range(N_CORES):
        r = results[c]
        out[0:1536] += r["out_p1"].T.reshape(1536)
        out[1536:2048] += r["out_p2"].reshape(512)
        new_state[4 * c : 4 * c + 4] = r["nst"]
        new_conv[_core_channels(c)] = r["ncv"].transpose(1, 0, 2).reshape(1024, 3)
    return (
        out.reshape(1, 1, H),
        new_state.reshape(1, 32, 128, 128),
        new_conv.reshape(1, 8192, 3),
    )


def run_sharded(inputs, **run_kwargs):
    nc = _get_nc()
    in_maps = shard_inputs(**inputs)
    res = run_bass_kernel_spmd(nc, in_maps, list(range(N_CORES)), **run_kwargs)
    return combine_outputs(res.results), res


def kernel(**inputs):
    outs, _ = run_sharded(inputs)
    return outs
